# revision 4
# baseline (speedup 1.0000x reference)
"""DetConB loss (nn_DetConBLoss) on 8 TRN2 NeuronCores via Bass/Tile.

Strategy (data-parallel over batch, targets replicated):
  - Host: l2-normalize preds/targets in f32, flatten to (4096, 256),
    transpose to (d, rows), cast fp8, k-interleave per 512-col block so
    every DMA is contiguous and every DoubleRow matmul AP is a dense 3D
    slice. Core c owns pred rows [c*512, (c+1)*512); targets arrive with
    columns rolled by c*512 (SPMD-identical program).
  - Device (per core): 16 iterations (tsel, px, mt) x 2 half-tiles of
    (128 rows x 2048 target cols). Per half: 4 fp8 DoubleRow matmuls
    (K=256 in one pass) into a double-buffered PSUM tile; the row-sum of
    exp(scale*logits) is split between ScalarE (ACTIVATE exp on cols
    0:1280 with its free accumulator) and VectorE (Schraudolph fast-exp
    int32 bit-trick on cols 1280:2048, then a fused
    scalar_tensor_tensor fold+accumulate over the staged halves). Only
    the 32 KB strip of row-sum partials leaves the device.
  - Host: 16x16 own-image diagonal dot blocks (recomputed from the same
    fp8 inputs, ~0.4% of FLOPs), roi masks, positive-pair sums, the
    -inf masking correction, log, and the final mean.
"""
import numpy as np
import ml_dtypes

import concourse.bacc as bacc
import concourse.mybir as mybir
import concourse.tile as tile
from concourse.bass_utils import run_bass_kernel_spmd

TEMP = 0.1
EPS = 1e-11
SCALE = float(np.float32(1.0 / (TEMP + EPS)))
NCORES = 8
B, N, D = 256, 16, 256
R = B * N          # 4096 flat rows
RPC = R // NCORES  # 512 rows per core
BF16 = mybir.dt.bfloat16
FP8 = mybir.dt.float8e4
NPFP8 = ml_dtypes.float8_e4m3
F32 = mybir.dt.float32
I32 = mybir.dt.int32
# Schraudolph fast-exp: exp(s*x) ~= bitcast_f32(int32(x*SA + SB))
SA = float(np.float32((2**23 / np.log(2.0)) * (1.0 / (0.1 + 1e-11))))
SB = float(np.float32(127 * 2**23 - 486411))

XA = 1280          # cols per half handled by ACT exp+accum
XZ = 2048 - XA     # cols per half handled by DVE schraudolph
NH = 2             # halves per iteration


def build_nc():
    """Build + schedule + compile the SPMD per-core Bass program."""
    nc = bacc.Bacc("TRN2", target_bir_lowering=False, debug=False,
                   num_devices=NCORES)

    # k-interleaved layouts: p [128, mt(4) * k(2) * 128], t [128, blk(8) *
    # k(2) * 512] fp8.
    p_dram = [nc.dram_tensor(f"p{i + 1}t", [128, 1024], FP8,
                             kind="ExternalInput") for i in range(2)]
    t_dram = [nc.dram_tensor(f"t{i + 1}t", [128, 8192], FP8,
                             kind="ExternalInput") for i in range(2)]
    sacc = nc.dram_tensor("sacc", [128, 64], F32, kind="ExternalOutput")

    with tile.TileContext(nc) as tc:
        with (
            tc.tile_pool(name="const", bufs=1) as const_pool,
            tc.tile_pool(name="psum", bufs=2, space="PSUM") as psum_pool,
            tc.tile_pool(name="scr", bufs=2) as scr_pool,
            tc.tile_pool(name="stage", bufs=2) as stage_pool,
        ):
            t_sb = [const_pool.tile([128, 8192], FP8, name=f"t_sb{i}",
                                    tag=f"t{i}") for i in range(2)]
            p_sb = [const_pool.tile([128, 1024], FP8, name=f"p_sb{i}",
                                    tag=f"p{i}") for i in range(2)]
            strip = const_pool.tile([128, 64], F32, name="strip", tag="strip")
            zbias = const_pool.tile([128, 1], F32, name="zbias", tag="zbias")
            warm = const_pool.tile([128, 2], BF16, name="warm", tag="warm")
            nc.vector.memset(zbias, 0.0)
            nc.vector.memset(warm, 0.0)
            # Warm the exp table set (ACT queue) before the first real
            # ACTIVATE so it does not pay the ~2.7us ACT_TABLE_LOAD.
            nc.scalar.activation(warm, warm,
                                 mybir.ActivationFunctionType.Exp,
                                 bias=zbias)

            # Input DMAs. sync (HWDGE) carries the critical path: p1 and
            # t1; scalar (also HWDGE on TRN2) carries p2 + the first t2
            # chunk; remaining t2 chunks are issued inside the loop from
            # the scalar queue to fill its idle slots.
            nc.sync.dma_start(out=p_sb[0], in_=p_dram[0].ap())
            nc.sync.dma_start(out=t_sb[0][:, 0:1024],
                              in_=t_dram[0][:, 0:1024])
            nc.sync.dma_start(out=t_sb[0][:, 1024:2048],
                              in_=t_dram[0][:, 1024:2048])
            nc.sync.dma_start(out=t_sb[0][:, 2048:4096],
                              in_=t_dram[0][:, 2048:4096])
            nc.sync.dma_start(out=t_sb[0][:, 4096:6144],
                              in_=t_dram[0][:, 4096:6144])
            nc.sync.dma_start(out=t_sb[0][:, 6144:8192],
                              in_=t_dram[0][:, 6144:8192])
            nc.scalar.dma_start(out=p_sb[1], in_=p_dram[1].ap())
            # t2 chunks deferred into the loop (4 x 2048 fp8 cols each).
            t2_chunks = [(i * 2048, (i + 1) * 2048) for i in range(4)]

            for tsel in range(2):
                for px in range(2):
                    for mt in range(4):
                        it = tsel * 8 + px * 4 + mt
                        lhs = p_sb[px][:, mt * 256:(mt + 1) * 256].rearrange(
                            "p (k c) -> p k c", k=2)
                        for h in range(2):
                            ps = psum_pool.tile([128, 2048], F32,
                                                name=f"ps{h}", tag="ps")
                            for jj in range(4):
                                blk = 4 * h + jj
                                rhs = t_sb[tsel][
                                    :, blk * 1024:(blk + 1) * 1024
                                ].rearrange("p (k c) -> p k c", k=2)
                                nc.tensor.matmul(
                                    ps[:, jj * 512:(jj + 1) * 512], lhs, rhs,
                                    start=True, stop=True,
                                    perf_mode=mybir.MatmulPerfMode.DoubleRow)
                            c0 = 4 * it + 2 * h
                            scr = scr_pool.tile([128, XA], BF16,
                                                name=f"scr{h}", tag="scr")
                            nc.scalar.activation(
                                scr, ps[:, 0:XA],
                                mybir.ActivationFunctionType.Exp,
                                bias=zbias, scale=SCALE,
                                accum_out=strip[:, c0:c0 + 1])
                            stage = stage_pool.tile([128, XZ], I32,
                                                    name=f"stage{h}", tag="stg")
                            nc.vector.tensor_scalar(
                                stage, ps[:, XA:2048], SA, SB,
                                op0=mybir.AluOpType.mult,
                                op1=mybir.AluOpType.add)
                            stf = stage.bitcast(F32)
                            dum = scr_pool.tile([128, XZ // 2], F32,
                                                name=f"dum{h}", tag="dum")
                            nc.vector.scalar_tensor_tensor(
                                dum, stf[:, 0:XZ // 2], 1.0,
                                stf[:, XZ // 2:XZ],
                                op0=mybir.AluOpType.mult,
                                op1=mybir.AluOpType.add,
                                accum_out=strip[:, c0 + 1:c0 + 2])
                        if tsel == 0 and px == 0 and t2_chunks:
                            a, b = t2_chunks.pop(0)
                            nc.scalar.dma_start(out=t_sb[1][:, a:b],
                                                in_=t_dram[1][:, a:b])
            nc.sync.dma_start(out=sacc.ap(), in_=strip)

    nc.compile()
    return nc


_NC = None


def _get_nc():
    global _NC
    if _NC is None:
        _NC = build_nc()
    return _NC


def _l2norm(x):
    return x / np.linalg.norm(x, axis=-1, keepdims=True)


def _swizzle_p(pt):
    """[D=256, 512] fp8 -> [128, mt(4) x k(2) x 128] contiguous."""
    return np.ascontiguousarray(
        pt.reshape(2, 128, 4, 128).transpose(1, 2, 0, 3).reshape(128, 1024))


def _swizzle_t(tt):
    """[D=256, 4096] fp8 -> [128, blk(8) x k(2) x 512] contiguous."""
    return np.ascontiguousarray(
        tt.reshape(2, 128, 8, 512).transpose(1, 2, 0, 3).reshape(128, 8192))


def host_prep(pred1, pred2, target1, target2):
    p1t = _l2norm(np.asarray(pred1, np.float32)).reshape(R, D).T.astype(NPFP8)
    p2t = _l2norm(np.asarray(pred2, np.float32)).reshape(R, D).T.astype(NPFP8)
    t1t = _l2norm(np.asarray(target1, np.float32)).reshape(R, D).T.astype(NPFP8)
    t2t = _l2norm(np.asarray(target2, np.float32)).reshape(R, D).T.astype(NPFP8)
    # Raw own-image diagonal dot blocks (b, n, m), fp8-quantized operands in
    # f32 — the same products the device computes, ~0.4% of total FLOPs.
    pf = [p1t.T.astype(np.float32).reshape(B, N, D),
          p2t.T.astype(np.float32).reshape(B, N, D)]
    tf = [t1t.T.astype(np.float32).reshape(B, N, D),
          t2t.T.astype(np.float32).reshape(B, N, D)]
    diag = [[np.einsum('bnd,bmd->bnm', pf[px], tf[ts]).astype(np.float32)
             for ts in range(2)] for px in range(2)]
    in_maps = []
    for c in range(NCORES):
        r0 = c * RPC
        t1r = np.concatenate([t1t[:, r0:], t1t[:, :r0]], axis=1)
        t2r = np.concatenate([t2t[:, r0:], t2t[:, :r0]], axis=1)
        in_maps.append({
            "p1t": _swizzle_p(p1t[:, r0:r0 + RPC]),
            "p2t": _swizzle_p(p2t[:, r0:r0 + RPC]),
            "t1t": _swizzle_t(t1r),
            "t2t": _swizzle_t(t2r),
        })
    return in_maps, diag


def host_post(results, diag, pind1, pind2, tind1, tind2):
    S = np.zeros((2, R), np.float64)
    for c, res in enumerate(results):
        sacc = np.asarray(res["sacc"])
        for px in range(2):
            for mt in range(4):
                r0 = c * RPC + mt * 128
                cols = [4 * (tsel * 8 + px * 4 + mt) + j
                        for tsel in range(2) for j in range(4)]
                S[px, r0:r0 + 128] = sacc[:, cols].astype(np.float64).sum(axis=1)
    sc = np.float32(SCALE)
    D_aa = sc * diag[0][0]
    D_ab = sc * diag[0][1]
    D_ba = sc * diag[1][0]
    D_bb = sc * diag[1][1]

    f32 = np.float32
    pind1, pind2 = np.asarray(pind1), np.asarray(pind2)
    tind1, tind2 = np.asarray(tind1), np.asarray(tind2)
    same_aa = (pind1[:, :, None] == tind1[:, None, :]).astype(f32)
    same_ab = (pind1[:, :, None] == tind2[:, None, :]).astype(f32)
    same_ba = (pind2[:, :, None] == tind1[:, None, :]).astype(f32)
    same_bb = (pind2[:, :, None] == tind2[:, None, :]).astype(f32)

    S0 = S[0].reshape(B, N)
    S1 = S[1].reshape(B, N)
    corr0 = (same_aa * np.exp(D_aa.astype(np.float64))).sum(-1)
    corr1 = (same_bb * np.exp(D_bb.astype(np.float64))).sum(-1)
    lse0 = np.log(S0 - corr0)
    lse1 = np.log(S1 - corr1)

    num_pos0 = same_ab.sum(-1)
    num_pos1 = same_ba.sum(-1)
    pos_sum0 = (same_ab * D_ab).sum(-1)
    pos_sum1 = (same_ba * D_ba).sum(-1)

    area0 = (pind1[:, :, None] == pind1[:, None, :]).astype(f32).sum(-1)
    area1 = (pind2[:, :, None] == pind2[:, None, :]).astype(f32).sum(-1)
    w0 = (num_pos0 > 0.001).astype(f32) / area0
    w1 = (num_pos1 > 0.001).astype(f32) / area1

    ce0 = -w0 * (pos_sum0 - num_pos0 * lse0) / np.maximum(num_pos0, 1.0)
    ce1 = -w1 * (pos_sum1 - num_pos1 * lse1) / np.maximum(num_pos1, 1.0)
    return np.float32(ce0.mean() + ce1.mean())


def run_hw(inputs, trace=False):
    nc = _get_nc()
    in_maps, diag = host_prep(inputs["pred1"], inputs["pred2"],
                              inputs["target1"], inputs["target2"])
    last_err = None
    for attempt in range(3):
        try:
            res = run_bass_kernel_spmd(nc, in_maps,
                                       core_ids=list(range(NCORES)),
                                       trace=trace)
            break
        except Exception as e:  # transient NRT device errors recover on retry
            last_err = e
            import time
            time.sleep(20 * (attempt + 1))
    else:
        raise last_err
    loss = host_post(res.results, diag, inputs["pind1"], inputs["pind2"],
                     inputs["tind1"], inputs["tind2"])
    return loss, res


def kernel(**inputs):
    loss, _ = run_hw(inputs, trace=False)
    return loss


# revision 5
# speedup vs baseline: 1.0511x; 1.0511x over previous
"""DetConB loss (nn_DetConBLoss) on 8 TRN2 NeuronCores via Bass/Tile.

Strategy (data-parallel over batch, targets replicated):
  - Host: l2-normalize preds/targets in f32, flatten to (4096, 256),
    transpose to (d, rows), cast fp8, k-interleave per 512-col block so
    every DMA is contiguous and every DoubleRow matmul AP is a dense 3D
    slice. Core c owns pred rows [c*512, (c+1)*512); targets arrive with
    columns rolled by c*512 (SPMD-identical program).
  - Device (per core): 16 iterations (tsel, px, mt) x 2 half-tiles of
    (128 rows x 2048 target cols). Per half: 4 fp8 DoubleRow matmuls
    (K=256 in one pass) into a double-buffered PSUM tile; the row-sum of
    exp(scale*logits) is split between ScalarE (ACTIVATE exp on cols
    0:1280 with its free accumulator) and VectorE (Schraudolph fast-exp
    int32 bit-trick on cols 1280:2048, then a fused
    scalar_tensor_tensor fold+accumulate over the staged halves). Only
    the 32 KB strip of row-sum partials leaves the device.
  - Host: 16x16 own-image diagonal dot blocks (recomputed from the same
    fp8 inputs, ~0.4% of FLOPs), roi masks, positive-pair sums, the
    -inf masking correction, log, and the final mean.
"""
import numpy as np
import ml_dtypes

import concourse.bacc as bacc
import concourse.mybir as mybir
import concourse.tile as tile
from concourse.bass_utils import run_bass_kernel_spmd

TEMP = 0.1
EPS = 1e-11
SCALE = float(np.float32(1.0 / (TEMP + EPS)))
NCORES = 8
B, N, D = 256, 16, 256
R = B * N          # 4096 flat rows
RPC = R // NCORES  # 512 rows per core
BF16 = mybir.dt.bfloat16
FP8 = mybir.dt.float8e4
NPFP8 = ml_dtypes.float8_e4m3
F32 = mybir.dt.float32
I32 = mybir.dt.int32
# Schraudolph fast-exp: exp(s*x) ~= bitcast_f32(int32(x*SA + SB))
SA = float(np.float32((2**23 / np.log(2.0)) * (1.0 / (0.1 + 1e-11))))
SB = float(np.float32(127 * 2**23 - 486411))

XZ = 640           # cols per half handled by DVE schraudolph (front)
XA = 2048 - XZ     # cols per half handled by ACT exp+accum
NH = 2             # halves per iteration


def build_nc():
    """Build + schedule + compile the SPMD per-core Bass program."""
    nc = bacc.Bacc("TRN2", target_bir_lowering=False, debug=False,
                   num_devices=NCORES)

    # k-interleaved layouts: p [128, mt(4) * k(2) * 128], t [128, blk(8) *
    # k(2) * 512] fp8.
    p_dram = [nc.dram_tensor(f"p{i + 1}t", [128, 1024], FP8,
                             kind="ExternalInput") for i in range(2)]
    t_dram = [nc.dram_tensor(f"t{i + 1}t", [128, 8192], FP8,
                             kind="ExternalInput") for i in range(2)]
    sacc = nc.dram_tensor("sacc", [128, 64], F32, kind="ExternalOutput")

    with tile.TileContext(nc) as tc:
        with (
            tc.tile_pool(name="const", bufs=1) as const_pool,
            tc.tile_pool(name="psum", bufs=2, space="PSUM") as psum_pool,
            tc.tile_pool(name="scr", bufs=2) as scr_pool,
            tc.tile_pool(name="stage", bufs=2) as stage_pool,
        ):
            t_sb = [const_pool.tile([128, 8192], FP8, name=f"t_sb{i}",
                                    tag=f"t{i}") for i in range(2)]
            p_sb = [const_pool.tile([128, 1024], FP8, name=f"p_sb{i}",
                                    tag=f"p{i}") for i in range(2)]
            strip = const_pool.tile([128, 64], F32, name="strip", tag="strip")
            zbias = const_pool.tile([128, 1], F32, name="zbias", tag="zbias")
            warm = const_pool.tile([128, 2], BF16, name="warm", tag="warm")
            nc.vector.memset(zbias, 0.0)
            nc.vector.memset(warm, 0.0)

            # Input DMAs. sync (HWDGE) carries the critical path: p1 and
            # t1; scalar (also HWDGE on TRN2) carries p2 + the first t2
            # chunk; remaining t2 chunks are issued inside the loop from
            # the scalar queue to fill its idle slots.
            nc.sync.dma_start(out=p_sb[0], in_=p_dram[0].ap())
            nc.sync.dma_start(out=t_sb[0][:, 0:1024],
                              in_=t_dram[0][:, 0:1024])
            nc.sync.dma_start(out=t_sb[0][:, 1024:2048],
                              in_=t_dram[0][:, 1024:2048])
            nc.scalar.dma_start(out=t_sb[0][:, 2048:4096],
                              in_=t_dram[0][:, 2048:4096])
            nc.sync.dma_start(out=t_sb[0][:, 4096:6144],
                              in_=t_dram[0][:, 4096:6144])
            nc.sync.dma_start(out=t_sb[0][:, 6144:8192],
                              in_=t_dram[0][:, 6144:8192])
            nc.sync.dma_start(out=p_sb[1], in_=p_dram[1].ap())
            # Warm the exp table set (ACT queue, after its DMA issue) so
            # the first real ACTIVATE does not pay the ~2.7us table load.
            nc.scalar.activation(warm, warm,
                                 mybir.ActivationFunctionType.Exp,
                                 bias=zbias)
            # t2 chunks deferred into the loop (4 x 2048 fp8 cols each),
            # issued from the otherwise-idle sync queue.
            t2_chunks = [(i * 2048, (i + 1) * 2048) for i in range(4)]

            # HAM warm-up: 8 dummy DoubleRow matmuls from a zeroed scratch
            # keep the PE busy ~3.4us while input DMAs land, so the clock
            # gate is at 8/8 when the real matmuls start.
            scratch = const_pool.tile([128, 1024], FP8, name="scratch",
                                      tag="scratch")
            nc.vector.memset(scratch, 0.0)
            ps_warm = psum_pool.tile([128, 2048], F32, name="ps0", tag="ps")
            dl = scratch[:, 0:256].rearrange("p (k c) -> p k c", k=2)
            dr = scratch.rearrange("p (k c) -> p k c", k=2)
            for _ in range(8):
                nc.tensor.matmul(ps_warm[:, 0:512], dl, dr,
                                 start=True, stop=True,
                                 perf_mode=mybir.MatmulPerfMode.DoubleRow)

            for tsel in range(2):
                for px in range(2):
                    for mt in range(4):
                        it = tsel * 8 + px * 4 + mt
                        lhs = p_sb[px][:, mt * 256:(mt + 1) * 256].rearrange(
                            "p (k c) -> p k c", k=2)
                        for h in range(2):
                            ps = psum_pool.tile([128, 2048], F32,
                                                name=f"ps{h}", tag="ps")
                            for jj in range(4):
                                blk = 4 * h + jj
                                rhs = t_sb[tsel][
                                    :, blk * 1024:(blk + 1) * 1024
                                ].rearrange("p (k c) -> p k c", k=2)
                                nc.tensor.matmul(
                                    ps[:, jj * 512:(jj + 1) * 512], lhs, rhs,
                                    start=True, stop=True,
                                    perf_mode=mybir.MatmulPerfMode.DoubleRow)
                            c0 = 4 * it + 2 * h
                            scr = scr_pool.tile([128, XA], BF16,
                                                name=f"scr{h}", tag="scr")
                            nc.scalar.activation(
                                scr, ps[:, XZ:2048],
                                mybir.ActivationFunctionType.Exp,
                                bias=zbias, scale=SCALE,
                                accum_out=strip[:, c0:c0 + 1])
                            stage = stage_pool.tile([128, XZ], I32,
                                                    name=f"stage{h}", tag="stg")
                            nc.vector.tensor_scalar(
                                stage, ps[:, 0:XZ], SA, SB,
                                op0=mybir.AluOpType.mult,
                                op1=mybir.AluOpType.add)
                            stf = stage.bitcast(F32)
                            dum = scr_pool.tile([128, XZ // 2], F32,
                                                name=f"dum{h}", tag="dum")
                            nc.vector.scalar_tensor_tensor(
                                dum, stf[:, 0:XZ // 2], 1.0,
                                stf[:, XZ // 2:XZ],
                                op0=mybir.AluOpType.mult,
                                op1=mybir.AluOpType.add,
                                accum_out=strip[:, c0 + 1:c0 + 2])
                        if tsel == 0 and px == 0 and t2_chunks:
                            a, b = t2_chunks.pop(0)
                            nc.sync.dma_start(out=t_sb[1][:, a:b],
                                              in_=t_dram[1][:, a:b])
            nc.sync.dma_start(out=sacc.ap(), in_=strip)

    nc.compile()
    return nc


_NC = None


def _get_nc():
    global _NC
    if _NC is None:
        _NC = build_nc()
    return _NC


def _l2norm(x):
    return x / np.linalg.norm(x, axis=-1, keepdims=True)


def _swizzle_p(pt):
    """[D=256, 512] fp8 -> [128, mt(4) x k(2) x 128] contiguous."""
    return np.ascontiguousarray(
        pt.reshape(2, 128, 4, 128).transpose(1, 2, 0, 3).reshape(128, 1024))


def _swizzle_t(tt):
    """[D=256, 4096] fp8 -> [128, blk(8) x k(2) x 512] contiguous."""
    return np.ascontiguousarray(
        tt.reshape(2, 128, 8, 512).transpose(1, 2, 0, 3).reshape(128, 8192))


def host_prep(pred1, pred2, target1, target2):
    p1t = _l2norm(np.asarray(pred1, np.float32)).reshape(R, D).T.astype(NPFP8)
    p2t = _l2norm(np.asarray(pred2, np.float32)).reshape(R, D).T.astype(NPFP8)
    t1t = _l2norm(np.asarray(target1, np.float32)).reshape(R, D).T.astype(NPFP8)
    t2t = _l2norm(np.asarray(target2, np.float32)).reshape(R, D).T.astype(NPFP8)
    # Raw own-image diagonal dot blocks (b, n, m), fp8-quantized operands in
    # f32 — the same products the device computes, ~0.4% of total FLOPs.
    pf = [p1t.T.astype(np.float32).reshape(B, N, D),
          p2t.T.astype(np.float32).reshape(B, N, D)]
    tf = [t1t.T.astype(np.float32).reshape(B, N, D),
          t2t.T.astype(np.float32).reshape(B, N, D)]
    diag = [[np.einsum('bnd,bmd->bnm', pf[px], tf[ts]).astype(np.float32)
             for ts in range(2)] for px in range(2)]
    in_maps = []
    for c in range(NCORES):
        r0 = c * RPC
        t1r = np.concatenate([t1t[:, r0:], t1t[:, :r0]], axis=1)
        t2r = np.concatenate([t2t[:, r0:], t2t[:, :r0]], axis=1)
        in_maps.append({
            "p1t": _swizzle_p(p1t[:, r0:r0 + RPC]),
            "p2t": _swizzle_p(p2t[:, r0:r0 + RPC]),
            "t1t": _swizzle_t(t1r),
            "t2t": _swizzle_t(t2r),
        })
    return in_maps, diag


def host_post(results, diag, pind1, pind2, tind1, tind2):
    S = np.zeros((2, R), np.float64)
    for c, res in enumerate(results):
        sacc = np.asarray(res["sacc"])
        for px in range(2):
            for mt in range(4):
                r0 = c * RPC + mt * 128
                cols = [4 * (tsel * 8 + px * 4 + mt) + j
                        for tsel in range(2) for j in range(4)]
                S[px, r0:r0 + 128] = sacc[:, cols].astype(np.float64).sum(axis=1)
    sc = np.float32(SCALE)
    D_aa = sc * diag[0][0]
    D_ab = sc * diag[0][1]
    D_ba = sc * diag[1][0]
    D_bb = sc * diag[1][1]

    f32 = np.float32
    pind1, pind2 = np.asarray(pind1), np.asarray(pind2)
    tind1, tind2 = np.asarray(tind1), np.asarray(tind2)
    same_aa = (pind1[:, :, None] == tind1[:, None, :]).astype(f32)
    same_ab = (pind1[:, :, None] == tind2[:, None, :]).astype(f32)
    same_ba = (pind2[:, :, None] == tind1[:, None, :]).astype(f32)
    same_bb = (pind2[:, :, None] == tind2[:, None, :]).astype(f32)

    S0 = S[0].reshape(B, N)
    S1 = S[1].reshape(B, N)
    corr0 = (same_aa * np.exp(D_aa.astype(np.float64))).sum(-1)
    corr1 = (same_bb * np.exp(D_bb.astype(np.float64))).sum(-1)
    lse0 = np.log(S0 - corr0)
    lse1 = np.log(S1 - corr1)

    num_pos0 = same_ab.sum(-1)
    num_pos1 = same_ba.sum(-1)
    pos_sum0 = (same_ab * D_ab).sum(-1)
    pos_sum1 = (same_ba * D_ba).sum(-1)

    area0 = (pind1[:, :, None] == pind1[:, None, :]).astype(f32).sum(-1)
    area1 = (pind2[:, :, None] == pind2[:, None, :]).astype(f32).sum(-1)
    w0 = (num_pos0 > 0.001).astype(f32) / area0
    w1 = (num_pos1 > 0.001).astype(f32) / area1

    ce0 = -w0 * (pos_sum0 - num_pos0 * lse0) / np.maximum(num_pos0, 1.0)
    ce1 = -w1 * (pos_sum1 - num_pos1 * lse1) / np.maximum(num_pos1, 1.0)
    return np.float32(ce0.mean() + ce1.mean())


def run_hw(inputs, trace=False):
    nc = _get_nc()
    in_maps, diag = host_prep(inputs["pred1"], inputs["pred2"],
                              inputs["target1"], inputs["target2"])
    last_err = None
    for attempt in range(3):
        try:
            res = run_bass_kernel_spmd(nc, in_maps,
                                       core_ids=list(range(NCORES)),
                                       trace=trace)
            break
        except Exception as e:  # transient NRT device errors recover on retry
            last_err = e
            import time
            time.sleep(20 * (attempt + 1))
    else:
        raise last_err
    loss = host_post(res.results, diag, inputs["pind1"], inputs["pind2"],
                     inputs["tind1"], inputs["tind2"])
    return loss, res


def kernel(**inputs):
    loss, _ = run_hw(inputs, trace=False)
    return loss
